# revision 13
# baseline (speedup 1.0000x reference)
"""Multi-head attention (B=1, S=4096, D=768, H=12) on 8 Trainium2 NeuronCores.

Sharding: 4 head-groups x 2 sequence-halves. Core (g, s) computes heads
[3g, 3g+3) for query rows [2048*s, 2048*(s+1)). O-proj partials of the 4
head-groups are summed on the host; halves concatenated, bias added.

PE packing (vs the plain version):
 - scores are computed in the log2 domain (16*log2e folded into Wq) as
   K=64 matmuls row-tiled onto PE halves: h0 lives in SBUF partitions
   0-63, h1 in 64-127, and h2's keys are SPLIT by sequence half (keys
   0-2047 in partitions 0-63, keys 2048-4095 in 64-127).  Processing key
   blocks in (j, j+16) pairs makes every adjacent scores matmul pair
   row-disjoint, so the PE runs them concurrently (2x).
 - attnV col-tiles h0 (PSUM partitions 0-63) with h1 (64-127) in one
   bank; h2 goes to a second bank whose spare col-groups carry M=1/M=2
   "rider" matmuls that accumulate the softmax denominators
   (sum0@p64, sum1@p96, sum2 = p65+p97 via [zero|ones] stationaries).
 - exp runs on BOTH Scalar (Act) and Vector (DVE) engines: h0/h1 tiles
   use Act's Exp (scale=ln2/128, bias=lnC); h2 tiles use a custom DVE op
   that emits bf16-bit-pattern int16s directly (Schraudolph-style with a
   quadratic mantissa correction, max rel err ~0.5%).  The global factor
   C cancels in softmax normalization.
"""

import numpy as np
import ml_dtypes

import concourse.bass as bass
import concourse.mybir as mybir
import concourse.tile as tile

BF16 = mybir.dt.bfloat16
FP32 = mybir.dt.float32
I16 = mybir.dt.int16

D = 768            # model dim
HD = 64            # head dim
HPC = 3            # heads per core
DH = HPC * HD      # 192: head dims per core
SEQ = 4096         # full sequence (keys)
SQ = 2048          # query rows per core
CT = D // 128      # 6 contraction tiles for projections
QB = 512           # query block (matmul free dim)
KBLK = 128         # key block (PSUM partition dim)
NKB = SEQ // KBLK  # 32
NJ = NKB // 2      # 16 key-block pairs (j, j+16)
KT = 512           # k/v load block
NKT = SEQ // KT    # 8
NQB = SQ // QB     # 4

# log2-domain score scale folded into Wq: scores arrive as y = 128*log2(e)/8*qk
QSCALE = 16.0 * np.log2(np.e)

# custom DVE exp2 (int16-Schraudolph) constants; ROUND-mode fit
EXP_C0 = 1610612736.0          # 1.5*2^30 magic
EXP_C1 = 16309.121602576064    # 127*128 + gamma
EXP_ALPHA = 0.002546052895563998
EXP_BETA = 0.9927928582620607
EXP_LNC = 0.3457785336582607   # ln of the global factor C
# trunc-mode alternates (if HW convert truncates)
EXP_C1_T = 16309.717284942835
EXP_ALPHA_T = 0.002587963943369594
EXP_BETA_T = 0.9953534901142183
EXP_LNC_T = 0.34638357108445816

ACT_SCALE = float(np.log(2.0) / 128.0)

# Custom DVE ops fail walrus codegen in this toolchain ("ISA wrong length"
# in CoreV2GenImpl visitInstISA) — even stock dve_ops entries. Keep the
# machinery but default to Act-only exp.
USE_DVE_EXP = False
DVE_TRUNC = False  # flip if HW int16 convert truncates


def _exp2_op():
    """Register (once) and return the custom DVE exp2-bits op."""
    import concourse.dve_ops as dops
    from concourse.dve_spec import Spec, Src0, Src1, C0, C1, C2, lower
    from concourse.dve_uop import DveOpSpec

    name = "EXP2BITS_ANT"
    for op in dops.OPS:
        if op.name == name:
            return op

    y = Src0
    t = y + C0
    u = t - C0
    x = y - u
    u2 = u + C1
    G = x * ((C2 * x) + Src1)
    body = u2 + G

    def _ref(in0, in1, s0, s1, imm2):
        yp = np.asarray(in0).astype(np.float32)
        tt = (yp + np.float32(s0)).astype(np.float32)
        uu = (tt - np.float32(s0)).astype(np.float32)
        xx = (yp - uu).astype(np.float32)
        uv = (uu + np.float32(s1)).astype(np.float32)
        beta = np.asarray(in1, np.float32).reshape(yp.shape[0], -1)[:, :1]
        Gv = (xx * (np.float32(imm2) * xx + beta)).astype(np.float32)
        return (uv + Gv).astype(np.float32)

    spec = Spec(body=body, reference=_ref)
    row = dops._CUSTOM_DVE_ROW_BASE + len(dops.OPS)
    assert row < 0x20
    dops._SUB_OPCODE_FOR_NAME[name] = row
    op = dops.DveOp(name, spec, subdim=False, uops_sha={})
    for ver in ("v3", "v4"):
        uops = lower(spec, ver=ver)
        op.uops_sha[ver] = DveOpSpec(
            name=name, opcode=row, uops=uops, rd1_en=True).sha(ver)
    dops.OPS.append(op)
    return op


def _patch_tile_drain():
    """walrus accepts only one sync-wait per CTRL instruction; split the
    TileContext exit's packed waits onto single-wait SP NOPs."""
    import bass_rust
    from concourse.vector_clock import ScopedClock

    def _split_drain_and_barrier(self, tick_clock, wait_clock):
        nc = self.nc
        probe = nc.sync.nop(nofuse=True)
        wait_clock.add_sem_waits(
            probe.ins, ScopedClock({None: tick_clock.global_clock})
        )
        si = probe.ins.sync_info
        waits = list(si.on_wait) if si is not None and si.on_wait else []
        if len(waits) > 1:
            probe.ins.sync_info = bass_rust.SyncInfo(
                on_wait=[waits[0]], on_update=[]
            )
            for w in waits[1:]:
                n = nc.sync.nop(nofuse=True)
                n.ins.sync_info = bass_rust.SyncInfo(on_wait=[w], on_update=[])
        nc.sync.drain()
        nc.all_engine_barrier()
        assert self.sems is not None
        popped = nc._tile_sem_poison_stack.pop()
        assert popped is self._sem_poison
        nc.clear_and_free_semaphores(list(self.sems.allocated().values()))
        nc.all_engine_barrier()

    tile.TileContext._drain_and_barrier = _split_drain_and_barrier


def _split_multi_waits(nc):
    """Hoist all-but-one sync-waits of every instruction onto preceding
    single-wait NOPs on the same engine (walrus 1-wait limit)."""
    import bass_rust
    n_split = 0
    for bb in nc.main_func.blocks:
        insts = bb.instructions
        new_list = []
        for inst in insts:
            si = getattr(inst, "sync_info", None)
            if si is not None and si.on_wait and len(si.on_wait) > 1:
                waits = list(si.on_wait)
                n_split += 1
                for w in waits[:-1]:
                    nop = mybir.InstNoOp(
                        name=nc.get_next_instruction_name(),
                        engine=inst.engine, ins=[], outs=[],
                        sync_info=bass_rust.SyncInfo(
                            on_wait=[w], on_update=[]))
                    new_list.append(nop)
                inst.sync_info = bass_rust.SyncInfo(
                    on_wait=[waits[-1]], on_update=list(si.on_update))
            new_list.append(inst)
        insts[:] = new_list
    return n_split


def build_program(has_bq: bool, has_bk: bool, has_bv: bool,
                  repeat: int = 1, qk_dtype=BF16) -> bass.Bass:
    _patch_tile_drain()
    nc = bass.Bass()

    qTs = nc.dram_tensor("qTs", [D, SQ], BF16, kind="ExternalInput")
    kT = nc.dram_tensor("kT", [D, SEQ], BF16, kind="ExternalInput")
    vT = nc.dram_tensor("vT", [D, SEQ], BF16, kind="ExternalInput")
    wq = nc.dram_tensor("wq", [D, DH], BF16, kind="ExternalInput")
    wk = nc.dram_tensor("wk", [D, DH], BF16, kind="ExternalInput")
    wv = nc.dram_tensor("wv", [D, DH], BF16, kind="ExternalInput")
    wo = nc.dram_tensor("wo", [DH, D], BF16, kind="ExternalInput")
    bqd = nc.dram_tensor("bq", [DH, 1], FP32, kind="ExternalInput")
    bkd = nc.dram_tensor("bk", [DH, 1], FP32, kind="ExternalInput")
    bvd = nc.dram_tensor("bv", [DH, 1], FP32, kind="ExternalInput")
    outT = nc.dram_tensor("outT", [D, SQ], FP32, kind="ExternalOutput")

    with tile.TileContext(nc) as tc:
        with (
            tc.tile_pool(name="persist", bufs=1) as persist,
            tc.tile_pool(name="small", bufs=2) as small,
        ):
            # persistent SBUF tensors
            khT_pair = persist.tile([128, SEQ], qk_dtype, tag="khp", name="khp")
            # h2 keys split by half: rows 0-63 = keys 0:2048, 64-127 = 2048:4096
            kh2 = persist.tile([128, SEQ // 2], qk_dtype, tag="kh2", name="kh2")
            qhT_pair = persist.tile([128, SQ], qk_dtype, tag="qhp", name="qhp")
            # h2 queries duplicated in both partition halves
            qh2 = persist.tile([128, SQ], qk_dtype, tag="qh2", name="qh2")
            vhx = [persist.tile([128, NKB * HD], BF16, tag=f"vhx{h}",
                                name=f"vhx{h}") for h in range(HPC)]
            wq_sb = persist.tile([128, CT * DH], BF16, tag="wq", name="wq_sb")
            wk_sb = persist.tile([128, CT * DH], BF16, tag="wk", name="wk_sb")
            wv_sb = persist.tile([128, CT * DH], BF16, tag="wv", name="wv_sb")
            wo_sb1 = persist.tile([128, D], BF16, tag="wo1", name="wo1")
            wo_sb2 = persist.tile([64, D], BF16, tag="wo2", name="wo2")
            bq_sb = persist.tile([128, 1], FP32, tag="bq1", name="bq1")
            bq2_sb = persist.tile([64, 1], FP32, tag="bq2", name="bq2")
            bk_sb = persist.tile([128, 1], FP32, tag="bk1", name="bk1")
            bk2_sb = persist.tile([64, 1], FP32, tag="bk2", name="bk2")
            bv_sb = persist.tile([64, HPC], FP32, tag="bv", name="bv_sb")
            ones_sb = persist.tile([1, 64], FP32, tag="ones", name="ones_sb")
            ones_col = persist.tile([128, 1], BF16, tag="onec", name="ones_col")
            beta_sb = persist.tile([128, 1], FP32, tag="beta", name="beta_sb")
            abias_sb = persist.tile([128, 1], FP32, tag="abias",
                                    name="abias_sb")

            nc.vector.memset(ones_sb[:], 1.0)
            nc.gpsimd.memset(ones_col[:], 1.0)
            nc.gpsimd.memset(
                beta_sb[:], EXP_BETA_T if DVE_TRUNC else EXP_BETA)
            nc.gpsimd.memset(
                abias_sb[:], EXP_LNC_T if DVE_TRUNC else EXP_LNC)

            persist_tiles = (khT_pair, kh2, qhT_pair, qh2, vhx,
                             wq_sb, wk_sb, wv_sb, wo_sb1, wo_sb2,
                             bq_sb, bq2_sb, bk_sb, bk2_sb, bv_sb, ones_sb,
                             ones_col, beta_sb, abias_sb,
                             qTs, kT, vT, outT,
                             wq, wk, wv, wo, bqd, bkd, bvd)
            for _rep in range(repeat):
                _phases(nc, tc, has_bq, has_bk, has_bv, persist_tiles, small)
    _split_multi_waits(nc)
    return nc


def _phases(nc, tc, has_bq, has_bk, has_bv, P, small):
    (khT_pair, kh2, qhT_pair, qh2, vhx, wq_sb, wk_sb, wv_sb,
     wo_sb1, wo_sb2, bq_sb, bq2_sb, bk_sb, bk2_sb, bv_sb, ones_sb,
     ones_col, beta_sb, abias_sb,
     qTs, kT, vT, outT, wq, wk, wv, wo, bqd, bkd, bvd) = P
    Exp = mybir.ActivationFunctionType.Exp
    exp_op = _exp2_op() if USE_DVE_EXP else None
    dve_c1 = EXP_C1_T if DVE_TRUNC else EXP_C1
    dve_alpha = EXP_ALPHA_T if DVE_TRUNC else EXP_ALPHA

    def psum_to_sbuf(dst_ap, src_ap, bias_ap):
        if bias_ap is None:
            nc.vector.tensor_copy(dst_ap, src_ap)
        else:
            nc.vector.tensor_scalar_add(dst_ap, src_ap, bias_ap)

    def kh_ap(h, kb):
        """lhsT AP for head h, key block kb (128 keys)."""
        if h == 0:
            return khT_pair[0:64, kb * KBLK:(kb + 1) * KBLK]
        if h == 1:
            return khT_pair[64:128, kb * KBLK:(kb + 1) * KBLK]
        # h2: keys 0:2048 in rows 0-63, 2048:4096 in rows 64-127
        if kb < NJ:
            return kh2[0:64, kb * KBLK:(kb + 1) * KBLK]
        return kh2[64:128, (kb - NJ) * KBLK:(kb - NJ + 1) * KBLK]

    def qh_ap(h, kb, q0, width):
        if h == 0:
            return qhT_pair[0:64, q0:q0 + width]
        if h == 1:
            return qhT_pair[64:128, q0:q0 + width]
        if kb < NJ:
            return qh2[0:64, q0:q0 + width]
        return qh2[64:128, q0:q0 + width]

    def exp_tile(pt_ap, sc_ap, on_dve):
        """pt (bf16) = C * 2^(sc/128), via Act or the custom DVE op."""
        if on_dve and exp_op is not None:
            nc.vector._custom_dve(
                exp_op, out=pt_ap.bitcast(I16), in0=sc_ap,
                in1=beta_sb[:, 0:1], s0=EXP_C0, s1=dve_c1, imm2=dve_alpha)
        else:
            nc.scalar.activation(pt_ap, sc_ap, Exp,
                                 bias=abias_sb[:, 0:1], scale=ACT_SCALE)

    def attention_j(j, qb, q0, scpool, ptpool, bankP, bankQ, bankS):
        """scores + exp + attnV/riders for key-block pair (j, j+16)."""
        kbA, kbB = j, j + NJ
        first, last = (j == 0), (j == NJ - 1)
        # --- scores: 6 MMs, row-half alternating L,U,L,U,(L||U) ---
        sc01 = scpool.tile([128, 2 * QB], FP32, tag="sc", name="sc01")
        sc01b = scpool.tile([128, 2 * QB], FP32, tag="sc", name="sc01b")
        nc.tensor.matmul(sc01[:, 0:QB], kh_ap(0, kbA),
                         qh_ap(0, kbA, q0, QB), start=True, stop=True)
        nc.tensor.matmul(sc01b[:, 0:QB], kh_ap(1, kbA),
                         qh_ap(1, kbA, q0, QB), start=True, stop=True)
        nc.tensor.matmul(sc01[:, QB:2 * QB], kh_ap(0, kbB),
                         qh_ap(0, kbB, q0, QB), start=True, stop=True)
        nc.tensor.matmul(sc01b[:, QB:2 * QB], kh_ap(1, kbB),
                         qh_ap(1, kbB, q0, QB), start=True, stop=True)
        pt0 = ptpool.tile([128, 2 * QB], BF16, tag="pt", name="pt0")
        exp_tile(pt0[:], sc01[:], on_dve=False)
        sc2 = scpool.tile([128, 2 * QB], FP32, tag="sc", name="sc2")
        # h2: kbA lives in rows 0-63, kbB in rows 64-127 -> concurrent pair
        nc.tensor.matmul(sc2[:, 0:QB], kh_ap(2, kbA),
                         qh_ap(2, kbA, q0, QB), start=True, stop=True)
        nc.tensor.matmul(sc2[:, QB:2 * QB], kh_ap(2, kbB),
                         qh_ap(2, kbB, q0, QB), start=True, stop=True)
        pt1 = ptpool.tile([128, 2 * QB], BF16, tag="pt", name="pt1")
        exp_tile(pt1[:], sc01b[:], on_dve=False)
        pt2 = ptpool.tile([128, 2 * QB], BF16, tag="pt", name="pt2")
        exp_tile(pt2[:], sc2[:], on_dve=USE_DVE_EXP)

        # --- attnV + denominator riders ---
        def vh(h, kb):
            return vhx[h][:, kb * HD:(kb + 1) * HD]

        pts = {0: pt0, 1: pt1, 2: pt2}

        def half(pt, kb):
            return pt[:, 0:QB] if kb == kbA else pt[:, QB:2 * QB]

        for kb in (kbA, kbB):
            st = first and kb == kbA
            sp = last and kb == kbB
            # slotA: h0 (cols 0-63) || h1 (cols 64-127)
            nc.tensor.matmul(bankP[0:64, :], vh(0, kb), half(pt0, kb),
                             start=st, stop=sp)
            nc.tensor.matmul(bankP[64:128, :], vh(1, kb), half(pt1, kb),
                             start=st, stop=sp)
            # slotB: h2 (cols 0-63) || sum0@p64 (col 64) || sum1@p96 (col 96)
            nc.tensor.matmul(bankQ[0:64, :], vh(2, kb), half(pt2, kb),
                             start=st, stop=sp)
            nc.tensor.matmul(bankS[64:65, :], ones_col[:], half(pt0, kb),
                             start=first and kb == kbA, stop=sp,
                             skip_group_check=True)
            nc.tensor.matmul(bankS[96:97, :], ones_col[:], half(pt1, kb),
                             start=first and kb == kbA, stop=sp,
                             skip_group_check=True, tile_position=(0, 96))
        # slotC: sum2 halves at p0 (keys 0:2048) and p32 (keys 2048:4096)
        nc.tensor.matmul(bankS[0:1, :], ones_col[:], half(pt2, kbA),
                         start=first, stop=last, skip_group_check=True)
        nc.tensor.matmul(bankS[32:33, :], ones_col[:], half(pt2, kbB),
                         start=first, stop=last, skip_group_check=True,
                         tile_position=(0, 32))

    def normalize_oproj(bankP, bankQ, bankS, q0, attnsb, psopool, outsb,
                        tag="fin", tbufs=None):
        attn_pair = attnsb.tile([128, QB], BF16, tag="apair", name="apair")
        attn_h2 = attnsb.tile([64, QB], BF16, tag="ah2", name="ah2")
        # sums: h0 @ p64, h1 @ p96, h2 = p65 + p97 (two accumulated mms)
        srcs = {0: [bankS[64:65, :]], 1: [bankS[96:97, :]],
                2: [bankS[0:1, :], bankS[32:33, :]]}
        accs = {0: bankP[0:64, :], 1: bankP[64:128, :], 2: bankQ[0:64, :]}
        for h in range(HPC):
            rb_ps = psopool.tile([64, QB], FP32, tag=tag, name="rb_ps",
                                 bufs=tbufs)
            n = len(srcs[h])
            for i, src in enumerate(srcs[h]):
                sums = small.tile([1, QB], FP32, tag="sums", name="sums")
                nc.vector.tensor_copy(sums[:], src)
                nc.tensor.matmul(rb_ps[:], ones_sb[:], sums[:],
                                 start=(i == 0), stop=(i == n - 1))
            rb = small.tile([64, QB], FP32, tag="rb", name="rb")
            nc.vector.reciprocal(rb[:], rb_ps[:])
            dst = (attn_pair[h * 64:(h + 1) * 64, :]
                   if h < 2 else attn_h2[:])
            nc.vector.tensor_mul(dst, accs[h], rb[:])
            if has_bv:
                nc.vector.tensor_scalar_add(dst, dst, bv_sb[:, h:h + 1])
        for et in range(CT):
            e0 = et * 128
            pso = psopool.tile([128, QB], FP32, tag=tag, name="pso",
                               bufs=tbufs)
            nc.tensor.matmul(pso[:], wo_sb1[:, e0:e0 + 128],
                             attn_pair[:], start=True, stop=False)
            nc.tensor.matmul(pso[:], wo_sb2[:, e0:e0 + 128],
                             attn_h2[:], start=False, stop=True)
            osb = outsb.tile([128, QB], FP32, tag="osb", name="osb")
            nc.vector.tensor_copy(osb[:], pso[:])
            nc.sync.dma_start(outT[e0:e0 + 128, q0:q0 + QB], osb[:])

    # ---- weight loads ----
    for ct in range(CT):
        nc.sync.dma_start(wq_sb[:, ct * DH:(ct + 1) * DH],
                          wq[ct * 128:ct * 128 + 128, :])
    if has_bq:
        nc.sync.dma_start(bq_sb[:], bqd[0:128, :])
        nc.sync.dma_start(bq2_sb[:], bqd[128:DH, :])

    def load_wkv():
        for ct in range(CT):
            c0 = ct * 128
            nc.sync.dma_start(wk_sb[:, ct * DH:(ct + 1) * DH],
                              wk[c0:c0 + 128, :])
            nc.sync.dma_start(wv_sb[:, ct * DH:(ct + 1) * DH],
                              wv[c0:c0 + 128, :])
        if has_bk:
            nc.sync.dma_start(bk_sb[:], bkd[0:128, :])
            nc.sync.dma_start(bk2_sb[:], bkd[128:DH, :])

    def load_wo():
        nc.sync.dma_start(wo_sb1[:], wo[0:128, :])
        nc.sync.dma_start(wo_sb2[:], wo[128:DH, :])
        if has_bv:
            for h in range(HPC):
                nc.sync.dma_start(bv_sb[:, h:h + 1],
                                  bvd[h * HD:(h + 1) * HD, :])

    def kproj(kt, kt_tiles, ps_kh, ps_kh2):
        """k-proj for 512 keys starting at kt*KT; h2 goes to the psum
        half matching its kh2 partition half."""
        k0 = kt * KT
        for ct in range(CT):
            nc.tensor.matmul(
                ps_kh[:], wk_sb[:, ct * DH:ct * DH + 128],
                kt_tiles[ct][:], start=(ct == 0), stop=(ct == CT - 1))
        lo = kt < NKT // 2
        out2 = ps_kh2[0:64, :] if lo else ps_kh2[64:128, :]
        for ct in range(CT):
            nc.tensor.matmul(
                out2, wk_sb[:, ct * DH + 128:(ct + 1) * DH],
                kt_tiles[ct][:], start=(ct == 0), stop=(ct == CT - 1))
        psum_to_sbuf(khT_pair[:, k0:k0 + KT], ps_kh[:],
                     bk_sb[:, 0:1] if has_bk else None)
        dst2 = (kh2[0:64, k0:k0 + KT] if lo
                else kh2[64:128, k0 - SEQ // 2:k0 - SEQ // 2 + KT])
        psum_to_sbuf(dst2, out2, bk2_sb[:, 0:1] if has_bk else None)

    def vproj(kt, vt_tiles, pproj):
        for sj in range(KT // KBLK):
            kb = kt * (KT // KBLK) + sj
            ps_vh = pproj.tile([128, DH], FP32, tag="pv", name="psvh")
            for ct in range(CT):
                nc.tensor.matmul(
                    ps_vh[:], vt_tiles[ct][:, sj * KBLK:(sj + 1) * KBLK],
                    wv_sb[:, ct * DH:(ct + 1) * DH],
                    start=(ct == 0), stop=(ct == CT - 1))
            for h in range(HPC):
                nc.vector.tensor_copy(
                    vhx[h][:, kb * HD:(kb + 1) * HD],
                    ps_vh[:, h * HD:(h + 1) * HD])

    # ---- Phase A: projections interleaved with attention for qb 0 ----
    with (
        tc.tile_pool(name="acc0", bufs=1, space="PSUM") as acc0_pool,
        tc.tile_pool(name="pt0_pool", bufs=6) as pt0_pool,
        tc.tile_pool(name="attnsb", bufs=2) as attnsb,
        tc.tile_pool(name="outsb", bufs=3) as outsb,
    ):
      bankP0 = acc0_pool.tile([128, QB], FP32, tag="bP", name="bankP0", bufs=1)
      bankQ0 = acc0_pool.tile([128, QB], FP32, tag="bQ", name="bankQ0", bufs=1)
      bankS0 = acc0_pool.tile([128, QB], FP32, tag="bS", name="bankS0", bufs=1)
      with (
        tc.tile_pool(name="stream", bufs=2) as stream,
        tc.tile_pool(name="pproj", bufs=1, space="PSUM") as pproj,
        tc.tile_pool(name="sc0", bufs=1, space="PSUM") as sc0_pool,
      ):
        # q projection (all four query blocks)
        qt2_tiles = []
        for st in range(NQB):
            s0 = st * QB
            ps_q = pproj.tile([128, QB], FP32, tag="pk", name="psq")
            ps_q2 = pproj.tile([128, QB], FP32, tag="pk2", name="psq2")
            if st % 2 == 0:
                qt2_tiles = []
                for ct in range(CT):
                    t = stream.tile([128, 2 * QB], BF16, tag="qt", name="qt",
                                    bufs=12)
                    nc.sync.dma_start(
                        t[:], qTs[ct * 128:(ct + 1) * 128, s0:s0 + 2 * QB])
                    qt2_tiles.append(t)
            qhalf = slice((st % 2) * QB, (st % 2) * QB + QB)
            qt_tiles = [t[:, qhalf] for t in qt2_tiles]
            for ct in range(CT):
                nc.tensor.matmul(
                    ps_q[:], wq_sb[:, ct * DH:ct * DH + 128], qt_tiles[ct][:],
                    start=(ct == 0), stop=(ct == CT - 1))
            # h2 q-proj duplicated into both psum halves (col-tiled pairs)
            for ct in range(CT):
                w2 = wq_sb[:, ct * DH + 128:(ct + 1) * DH]
                nc.tensor.matmul(ps_q2[0:64, :], w2, qt_tiles[ct][:],
                                 start=(ct == 0), stop=(ct == CT - 1))
                nc.tensor.matmul(ps_q2[64:128, :], w2, qt_tiles[ct][:],
                                 start=(ct == 0), stop=(ct == CT - 1))
            psum_to_sbuf(qhT_pair[:, s0:s0 + QB], ps_q[:],
                         bq_sb[:, 0:1] if has_bq else None)
            psum_to_sbuf(qh2[0:64, s0:s0 + QB], ps_q2[0:64, :],
                         bq2_sb[:, 0:1] if has_bq else None)
            psum_to_sbuf(qh2[64:128, s0:s0 + QB], ps_q2[64:128, :],
                         bq2_sb[:, 0:1] if has_bq else None)
            if st == 0:
                load_wkv()

        # k/v projections in kt order [0,4,1,5,2,6,3,7]; after each (m, m+4)
        # pair, run attention j-groups 4m..4m+3 for query block 0.
        kv_tiles = {}
        for m in range(NKT // 2):
            if m == 1:
                load_wo()
            for kt in (m, m + NKT // 2):
                k0 = kt * KT
                kw, vw = [], []
                for ct in range(CT):
                    c0 = ct * 128
                    t = stream.tile([128, KT], BF16, tag="ktile",
                                    name="ktile", bufs=12)
                    nc.sync.dma_start(t[:], kT[c0:c0 + 128, k0:k0 + KT])
                    kw.append(t)
                    t = stream.tile([128, KT], BF16, tag="vtile",
                                    name="vtile", bufs=12)
                    nc.sync.dma_start(t[:], vT[c0:c0 + 128, k0:k0 + KT])
                    vw.append(t)
                kv_tiles[kt] = (kw, vw)
                ps_kh = pproj.tile([128, KT], FP32, tag="pk", name="pskh")
                ps_kh2 = pproj.tile([128, KT], FP32, tag="pk2", name="pskh2")
                kproj(kt, kw, ps_kh, ps_kh2)
                vproj(kt, vw, pproj)
                kv_tiles.pop(kt)
            for j in range(4 * m, 4 * m + 4):
                attention_j(j, 0, 0, sc0_pool, pt0_pool, bankP0[:], bankQ0[:], bankS0[:])
      with tc.tile_pool(name="pfin", bufs=2, space="PSUM") as pfin:
        normalize_oproj(bankP0[:], bankQ0[:], bankS0[:], 0, attnsb, pfin, outsb,
                        tag="fin", tbufs=2)

    # ---- Phase B: attention + o-proj for query blocks 1..3 ----
    with (
        tc.tile_pool(name="scpool", bufs=2, space="PSUM") as scpool,
        tc.tile_pool(name="accpool", bufs=1, space="PSUM") as accpool,
        tc.tile_pool(name="psopool", bufs=1, space="PSUM") as psopool,
        tc.tile_pool(name="ptpool", bufs=8) as ptpool,
        tc.tile_pool(name="attnsb", bufs=2) as attnsb,
        tc.tile_pool(name="outsb", bufs=3) as outsb,
    ):
        for qb in range(1, NQB):
            q0 = qb * QB
            bankP = accpool.tile([128, QB], FP32, tag="accP", name="bankP")
            bankQ = accpool.tile([128, QB], FP32, tag="accQ", name="bankQ")
            bankS = accpool.tile([128, QB], FP32, tag="accS", name="bankS")
            for j in range(NJ):
                attention_j(j, qb, q0, scpool, ptpool, bankP[:], bankQ[:],
                            bankS[:])
            normalize_oproj(bankP[:], bankQ[:], bankS[:], q0, attnsb,
                            psopool, outsb, tag="pso")


def prepare(q, k, v, Wq, bq, Wk, bk, Wv, bv, Wo, bo):
    """Host-side sharding: returns (in_maps for cores 0-7, bias flags)."""
    bf = ml_dtypes.bfloat16
    qT = np.ascontiguousarray(q[0].T).astype(bf)
    kTf = np.ascontiguousarray(k[0].T).astype(bf)
    vTf = np.ascontiguousarray(v[0].T).astype(bf)
    wqT = np.ascontiguousarray(np.asarray(Wq).T * QSCALE).astype(bf)
    wkT = np.ascontiguousarray(np.asarray(Wk).T).astype(bf)
    wvT = np.ascontiguousarray(np.asarray(Wv).T).astype(bf)
    woT = np.ascontiguousarray(np.asarray(Wo).T).astype(bf)
    bq = np.asarray(bq, np.float32) * np.float32(QSCALE)
    bk = np.asarray(bk, np.float32)
    bv = np.asarray(bv, np.float32)
    in_maps = []
    for core in range(8):
        g, s = divmod(core, 2)
        d0, d1 = g * DH, (g + 1) * DH
        in_maps.append({
            "qTs": np.ascontiguousarray(qT[:, s * SQ:(s + 1) * SQ]),
            "kT": kTf,
            "vT": vTf,
            "wq": np.ascontiguousarray(wqT[:, d0:d1]),
            "wk": np.ascontiguousarray(wkT[:, d0:d1]),
            "wv": np.ascontiguousarray(wvT[:, d0:d1]),
            "wo": np.ascontiguousarray(woT[d0:d1, :]),
            "bq": np.ascontiguousarray(bq[d0:d1]).reshape(DH, 1),
            "bk": np.ascontiguousarray(bk[d0:d1]).reshape(DH, 1),
            "bv": np.ascontiguousarray(bv[d0:d1]).reshape(DH, 1),
        })
    flags = (bool(np.any(bq)), bool(np.any(bk)), bool(np.any(bv)))
    return in_maps, flags


def combine(results, bo):
    """Host-side unsharding: sum o-proj partials per half, concat, add bo."""
    halves = []
    for s in range(2):
        acc = None
        for g in range(4):
            o = np.asarray(results[g * 2 + s]["outT"], np.float32)
            acc = o if acc is None else acc + o
        halves.append(acc.T)
    out = np.concatenate(halves, axis=0) + np.asarray(bo, np.float32)
    return np.ascontiguousarray(out).reshape(1, SEQ, D).astype(np.float32)


def kernel(q, k, v, Wq, bq, Wk, bk, Wv, bv, Wo, bo):
    from concourse.bass_utils import run_bass_kernel_spmd

    in_maps, flags = prepare(q, k, v, Wq, bq, Wk, bk, Wv, bv, Wo, bo)
    nc = build_program(*flags)
    last_err = None
    for _attempt in range(3):
        try:
            res = run_bass_kernel_spmd(nc, in_maps, list(range(8)))
            return combine(res.results, bo)
        except Exception as e:  # transient NRT/device wedges recover on retry
            last_err = e
            try:
                import jax
                jax.clear_caches()
                jax.extend.backend.clear_backends()
            except Exception:
                pass
    raise last_err


# revision 14
# speedup vs baseline: 1.5448x; 1.5448x over previous
"""Multi-head attention (B=1, S=4096, D=768, H=12) on 8 Trainium2 NeuronCores.

Sharding: 4 head-groups x 2 sequence-halves. Core (g, s) computes heads
[3g, 3g+3) for query rows [2048*s, 2048*(s+1)): it projects q for its rows,
k/v for its heads over the full sequence, runs softmax(QK^T/8)V for its
(heads, rows) block, and applies its slice of the output projection. The
o-proj partials of the 4 head-groups are summed on the host (the all-reduce
step of tensor-parallel attention), halves concatenated, bias added.

On-chip layout notes:
 - scores are built transposed ([keys, queries]) so the attn@V matmul can
   contract keys on the partition axis with no transposes anywhere.
 - the head pair (h0, h1) shares the 128-row PE array via row tiling
   (K=64 each); the odd head h2 runs in rows 0-63 alone.
 - exp row-sums come for free from the attn@V matmul: V is extended with a
   65th column of ones, so PSUM row 64 accumulates sum_k exp(score).
 - softmax uses no max-subtraction: |scores| < ~30 here, safe in fp32.
 - exp instructions are 1024 wide everywhere (two key blocks per
   activation) to amortize the Scalar engine's per-instruction overhead —
   the Act engine is the bottleneck of this kernel.
"""

import numpy as np
import ml_dtypes

import concourse.bass as bass
import concourse.mybir as mybir
import concourse.tile as tile

BF16 = mybir.dt.bfloat16
FP32 = mybir.dt.float32

D = 768            # model dim
HD = 64            # head dim
HPC = 3            # heads per core
DH = HPC * HD      # 192: head dims per core
SEQ = 4096         # full sequence (keys)
SQ = 2048          # query rows per core
CT = D // 128      # 6 contraction tiles for projections
QB = 512           # query block (matmul free dim)
NQB = SQ // QB     # 4
KBLK = 128         # key block (PSUM partition dim)
NKB = SEQ // KBLK  # 32
KT = 512           # k/v load superblock
NKT = SEQ // KT    # 8
SCALE = 1.0 / 8.0  # 1/sqrt(HD)


def _patch_tile_drain():
    """walrus here accepts only one sync-wait per CTRL instruction; the stock
    TileContext exit packs every outstanding wait onto a single SP Drain.
    Split them onto single-wait SP NOPs that precede the drain."""
    import bass_rust
    from concourse.vector_clock import ScopedClock

    def _split_drain_and_barrier(self, tick_clock, wait_clock):
        nc = self.nc
        probe = nc.sync.nop(nofuse=True)
        wait_clock.add_sem_waits(
            probe.ins, ScopedClock({None: tick_clock.global_clock})
        )
        si = probe.ins.sync_info
        waits = list(si.on_wait) if si is not None and si.on_wait else []
        if len(waits) > 1:
            probe.ins.sync_info = bass_rust.SyncInfo(
                on_wait=[waits[0]], on_update=[]
            )
            for w in waits[1:]:
                n = nc.sync.nop(nofuse=True)
                n.ins.sync_info = bass_rust.SyncInfo(on_wait=[w], on_update=[])
        nc.sync.drain()
        nc.all_engine_barrier()
        assert self.sems is not None
        popped = nc._tile_sem_poison_stack.pop()
        assert popped is self._sem_poison
        nc.clear_and_free_semaphores(list(self.sems.allocated().values()))
        nc.all_engine_barrier()

    tile.TileContext._drain_and_barrier = _split_drain_and_barrier



def _split_multi_waits(nc):
    """Hoist all-but-one sync-waits of every instruction onto preceding
    single-wait NOPs on the same engine (walrus 1-wait limit)."""
    import bass_rust
    n_split = 0
    for bb in nc.main_func.blocks:
        insts = bb.instructions
        new_list = []
        for inst in insts:
            si = getattr(inst, "sync_info", None)
            if si is not None and si.on_wait and len(si.on_wait) > 1:
                waits = list(si.on_wait)
                n_split += 1
                for w in waits[:-1]:
                    nop = mybir.InstNoOp(
                        name=nc.get_next_instruction_name(),
                        engine=inst.engine, ins=[], outs=[],
                        sync_info=bass_rust.SyncInfo(
                            on_wait=[w], on_update=[]))
                    new_list.append(nop)
                inst.sync_info = bass_rust.SyncInfo(
                    on_wait=[waits[-1]], on_update=list(si.on_update))
            new_list.append(inst)
        insts[:] = new_list
    return n_split

def build_program(has_bq: bool, has_bk: bool, has_bv: bool,
                  repeat: int = 1, qk_dtype=BF16) -> bass.Bass:
    _patch_tile_drain()
    nc = bass.Bass()

    qTs = nc.dram_tensor("qTs", [D, SQ], BF16, kind="ExternalInput")
    kT = nc.dram_tensor("kT", [D, SEQ], BF16, kind="ExternalInput")
    vT = nc.dram_tensor("vT", [D, SEQ], BF16, kind="ExternalInput")
    wq = nc.dram_tensor("wq", [D, DH], BF16, kind="ExternalInput")
    wk = nc.dram_tensor("wk", [D, DH], BF16, kind="ExternalInput")
    wv = nc.dram_tensor("wv", [D, DH], BF16, kind="ExternalInput")
    wo = nc.dram_tensor("wo", [DH, D], BF16, kind="ExternalInput")
    bqd = nc.dram_tensor("bq", [DH, 1], FP32, kind="ExternalInput")
    bkd = nc.dram_tensor("bk", [DH, 1], FP32, kind="ExternalInput")
    bvd = nc.dram_tensor("bv", [DH, 1], FP32, kind="ExternalInput")
    outT = nc.dram_tensor("outT", [D, SQ], FP32, kind="ExternalOutput")

    Exp = mybir.ActivationFunctionType.Exp

    with tile.TileContext(nc) as tc:
        with (
            tc.tile_pool(name="persist", bufs=1) as persist,
            tc.tile_pool(name="small", bufs=2) as small,
        ):
            # persistent SBUF tensors
            khT_pair = persist.tile([128, SEQ], qk_dtype, tag="khp", name="khp")
            khT_h2 = persist.tile([64, SEQ], qk_dtype, tag="kh2", name="kh2")
            qhT_pair = persist.tile([128, SQ], qk_dtype, tag="qhp", name="qhp")
            qhT_h2 = persist.tile([64, SQ], qk_dtype, tag="qh2", name="qh2")
            vhx = [persist.tile([128, NKB * 65], BF16, tag=f"vhx{h}", name=f"vhx{h}")
                   for h in range(HPC)]
            wq_sb = persist.tile([128, CT * DH], BF16, tag="wq", name="wq_sb")
            wk_sb = persist.tile([128, CT * DH], BF16, tag="wk", name="wk_sb")
            wv_sb = persist.tile([128, CT * DH], BF16, tag="wv", name="wv_sb")
            wo_sb1 = persist.tile([128, D], BF16, tag="wo1", name="wo1")
            wo_sb2 = persist.tile([64, D], BF16, tag="wo2", name="wo2")
            bq_sb = persist.tile([128, 1], FP32, tag="bq1", name="bq1")
            bq2_sb = persist.tile([64, 1], FP32, tag="bq2", name="bq2")
            bk_sb = persist.tile([128, 1], FP32, tag="bk1", name="bk1")
            bk2_sb = persist.tile([64, 1], FP32, tag="bk2", name="bk2")
            bv_sb = persist.tile([64, HPC], FP32, tag="bv", name="bv_sb")
            ones_sb = persist.tile([1, 64], FP32, tag="ones", name="ones_sb")

            # ones columns for the exp-sum trick (overwritten with vh below)
            for h in range(HPC):
                nc.gpsimd.memset(vhx[h][:], 1.0)
            nc.vector.memset(ones_sb[:], 1.0)

            persist_tiles = (khT_pair, khT_h2, qhT_pair, qhT_h2, vhx,
                             wq_sb, wk_sb, wv_sb, wo_sb1, wo_sb2,
                             bq_sb, bq2_sb, bk_sb, bk2_sb, bv_sb, ones_sb,
                             qTs, kT, vT, outT,
                             wq, wk, wv, wo, bqd, bkd, bvd)
            for _rep in range(repeat):
                _phases(nc, tc, has_bq, has_bk, has_bv, persist_tiles, small)
    _split_multi_waits(nc)
    return nc


def _phases(nc, tc, has_bq, has_bk, has_bv, P, small):
    (khT_pair, khT_h2, qhT_pair, qhT_h2, vhx, wq_sb, wk_sb, wv_sb,
     wo_sb1, wo_sb2, bq_sb, bq2_sb, bk_sb, bk2_sb, bv_sb, ones_sb,
     qTs, kT, vT, outT, wq, wk, wv, wo, bqd, bkd, bvd) = P
    Exp = mybir.ActivationFunctionType.Exp

    def psum_to_sbuf(dst_ap, src_ap, bias_ap):
        if bias_ap is None:
            nc.vector.tensor_copy(dst_ap, src_ap)
        else:
            nc.vector.tensor_scalar_add(dst_ap, src_ap, bias_ap)

    def scores_mms(ps_ap, h, kb, q0, width):
        """scores^T[kb block, q0:q0+width] for head h into PSUM ap."""
        ks = slice(kb * KBLK, (kb + 1) * KBLK)
        if h == 0:
            lhs, rhs = khT_pair[0:64, ks], qhT_pair[0:64, q0:q0 + width]
        elif h == 1:
            lhs, rhs = khT_pair[64:128, ks], qhT_pair[64:128, q0:q0 + width]
        else:
            lhs, rhs = khT_h2[:, ks], qhT_h2[:, q0:q0 + width]
        nc.tensor.matmul(ps_ap, lhs, rhs, start=True, stop=True)

    def normalize_oproj(accs, q0, attnsb, accpool, outsb, tag="acc",
                        tbufs=None):
        attn_pair = attnsb.tile([128, QB], BF16, tag="apair", name="apair")
        attn_h2 = attnsb.tile([64, QB], BF16, tag="ah2", name="ah2")
        for h in range(HPC):
            sums = small.tile([1, QB], FP32, tag="sums", name="sums")
            nc.vector.tensor_copy(sums[:], accs[h][64:65, :])
            rb_ps = accpool.tile([64, QB], FP32, tag=tag, name="rb_ps",
                                 bufs=tbufs)
            nc.tensor.matmul(rb_ps[:], ones_sb[:], sums[:],
                             start=True, stop=True)
            rb = small.tile([64, QB], FP32, tag="rb", name="rb")
            nc.vector.reciprocal(rb[:], rb_ps[:])
            dst = (attn_pair[h * 64:(h + 1) * 64, :]
                   if h < 2 else attn_h2[:])
            nc.vector.tensor_mul(dst, accs[h][0:64, :], rb[:])
            if has_bv:
                nc.vector.tensor_scalar_add(dst, dst, bv_sb[:, h:h + 1])
        for et in range(CT):
            e0 = et * 128
            pso = accpool.tile([128, QB], FP32, tag=tag, name="pso",
                               bufs=tbufs)
            nc.tensor.matmul(pso[:], wo_sb1[:, e0:e0 + 128],
                             attn_pair[:], start=True, stop=False)
            nc.tensor.matmul(pso[:], wo_sb2[:, e0:e0 + 128],
                             attn_h2[:], start=False, stop=True)
            osb = outsb.tile([128, QB], FP32, tag="osb", name="osb")
            nc.vector.tensor_copy(osb[:], pso[:])
            nc.sync.dma_start(outT[e0:e0 + 128, q0:q0 + QB], osb[:])

    # weight loads, ordered to unblock the pipeline front-to-back
    for ct in range(CT):
        nc.sync.dma_start(wq_sb[:, ct * DH:(ct + 1) * DH],
                          wq[ct * 128:ct * 128 + 128, :])
    if has_bq:
        nc.sync.dma_start(bq_sb[:], bqd[0:128, :])
        nc.sync.dma_start(bq2_sb[:], bqd[128:DH, :])

    def load_wkv():
        for ct in range(CT):
            c0 = ct * 128
            nc.sync.dma_start(wk_sb[:, ct * DH:(ct + 1) * DH],
                              wk[c0:c0 + 128, :])
            nc.sync.dma_start(wv_sb[:, ct * DH:(ct + 1) * DH],
                              wv[c0:c0 + 128, :])
        if has_bk:
            nc.sync.dma_start(bk_sb[:], bkd[0:128, :])
            nc.sync.dma_start(bk2_sb[:], bkd[128:DH, :])

    def load_wo():
        nc.sync.dma_start(wo_sb1[:], wo[0:128, :])
        nc.sync.dma_start(wo_sb2[:], wo[128:DH, :])
        if has_bv:
            for h in range(HPC):
                nc.sync.dma_start(bv_sb[:, h:h + 1],
                                  bvd[h * HD:(h + 1) * HD, :])

    # ---- Phase A+B0: projections interleaved with attention for qb 0 ----
    # PSUM budget (8 banks): pk/pk2/pv share a 3-bank projection set,
    # qb0 scores 2 banks ([128, 2QB] single-buffered), qb0 accumulators 3.
    with (
        tc.tile_pool(name="acc0", bufs=1, space="PSUM") as acc0_pool,
        tc.tile_pool(name="pt0", bufs=6) as pt0_pool,
        tc.tile_pool(name="attnsb", bufs=2) as attnsb,
        tc.tile_pool(name="outsb", bufs=3) as outsb,
      ):
      accs0 = [acc0_pool.tile([128, QB], FP32, tag=f"a0{h}", name="a0",
                              bufs=1)
               for h in range(HPC)]
      with (
        tc.tile_pool(name="stream", bufs=2) as stream,
        tc.tile_pool(name="pproj", bufs=1, space="PSUM") as pproj,
        tc.tile_pool(name="sc0", bufs=1, space="PSUM") as sc0_pool,
      ):
        # q projection (all four query blocks)
        qt2_tiles = []
        for st in range(NQB):
            s0 = st * QB
            ps_q = pproj.tile([128, QB], FP32, tag="pk", name="psq")
            ps_q2 = pproj.tile([64, QB], FP32, tag="pk2", name="psq2")
            if st % 2 == 0:
                qt2_tiles = []
                for ct in range(CT):
                    t = stream.tile([128, 2 * QB], BF16, tag="qt", name="qt",
                                    bufs=12)
                    nc.sync.dma_start(
                        t[:], qTs[ct * 128:(ct + 1) * 128, s0:s0 + 2 * QB])
                    qt2_tiles.append(t)
            qhalf = slice((st % 2) * QB, (st % 2) * QB + QB)
            qt_tiles = [t[:, qhalf] for t in qt2_tiles]
            for ct in range(CT):
                nc.tensor.matmul(
                    ps_q[:], wq_sb[:, ct * DH:ct * DH + 128], qt_tiles[ct][:],
                    start=(ct == 0), stop=(ct == CT - 1))
            for ct in range(CT):
                nc.tensor.matmul(
                    ps_q2[:], wq_sb[:, ct * DH + 128:(ct + 1) * DH],
                    qt_tiles[ct][:],
                    start=(ct == 0), stop=(ct == CT - 1))
            psum_to_sbuf(qhT_pair[:, s0:s0 + QB], ps_q[:],
                         bq_sb[:, 0:1] if has_bq else None)
            psum_to_sbuf(qhT_h2[:, s0:s0 + QB], ps_q2[:],
                         bq2_sb[:, 0:1] if has_bq else None)
            if st == 0:
                load_wkv()

        kt2_tiles = {}
        for kt in range(NKT):
            k0 = kt * KT
            if kt == 2:
                load_wo()
            # k/v loads come in 1024-wide tiles (2KB partition lines);
            # each serves two 512-key superblocks.
            if kt % 2 == 0:
                kw, vw = [], []
                for ct in range(CT):
                    c0 = ct * 128
                    t = stream.tile([128, 2 * KT], BF16, tag="ktile",
                                    name="ktile", bufs=12)
                    nc.sync.dma_start(t[:], kT[c0:c0 + 128, k0:k0 + 2 * KT])
                    kw.append(t)
                    t = stream.tile([128, 2 * KT], BF16, tag="vtile",
                                    name="vtile", bufs=12)
                    nc.sync.dma_start(t[:], vT[c0:c0 + 128, k0:k0 + 2 * KT])
                    vw.append(t)
                kt2_tiles = {"k": kw, "v": vw}
            half = slice((kt % 2) * KT, (kt % 2) * KT + KT)
            kt_tiles = [t[:, half] for t in kt2_tiles["k"]]
            vt_tiles = [t[:, half] for t in kt2_tiles["v"]]
            ps_kh = pproj.tile([128, KT], FP32, tag="pk", name="pskh")
            ps_kh2 = pproj.tile([64, KT], FP32, tag="pk2", name="pskh2")
            for ct in range(CT):
                nc.tensor.matmul(
                    ps_kh[:], wk_sb[:, ct * DH:ct * DH + 128],
                    kt_tiles[ct][:], start=(ct == 0), stop=(ct == CT - 1))
            for ct in range(CT):
                nc.tensor.matmul(
                    ps_kh2[:], wk_sb[:, ct * DH + 128:(ct + 1) * DH],
                    kt_tiles[ct][:], start=(ct == 0), stop=(ct == CT - 1))
            psum_to_sbuf(khT_pair[:, k0:k0 + KT], ps_kh[:],
                         bk_sb[:, 0:1] if has_bk else None)
            psum_to_sbuf(khT_h2[:, k0:k0 + KT], ps_kh2[:],
                         bk2_sb[:, 0:1] if has_bk else None)
            # v-proj for the 4 key blocks of this kt, then attention for
            # query block 0 in key-block PAIRS (1024-wide exps).
            for sj2 in range(KT // KBLK // 2):
                kbs = [kt * (KT // KBLK) + 2 * sj2 + i for i in range(2)]
                for kb in kbs:
                    sj = kb - kt * (KT // KBLK)
                    ps_vh = pproj.tile([128, DH], FP32, tag="pv", name="psvh")
                    for ct in range(CT):
                        nc.tensor.matmul(
                            ps_vh[:],
                            vt_tiles[ct][:, sj * KBLK:(sj + 1) * KBLK],
                            wv_sb[:, ct * DH:(ct + 1) * DH],
                            start=(ct == 0), stop=(ct == CT - 1))
                    for h in range(HPC):
                        nc.vector.tensor_copy(
                            vhx[h][:, kb * 65:kb * 65 + 64],
                            ps_vh[:, h * HD:(h + 1) * HD])
                for h in range(HPC):
                    sc = sc0_pool.tile([128, 2 * QB], FP32, tag="sc0",
                                       name="sc0")
                    for i, kb in enumerate(kbs):
                        scores_mms(sc[:, i * QB:(i + 1) * QB], h, kb, 0, QB)
                    pt = pt0_pool.tile([128, 2 * QB], BF16, tag="pt0",
                                       name="pt0")
                    nc.scalar.activation(pt[:], sc[:], Exp, scale=SCALE)
                    for i, kb in enumerate(kbs):
                        nc.tensor.matmul(
                            accs0[h][0:65, :],
                            vhx[h][:, kb * 65:kb * 65 + 65],
                            pt[:, i * QB:(i + 1) * QB],
                            start=(kb == 0), stop=(kb == NKB - 1))
      with tc.tile_pool(name="pfin", bufs=2, space="PSUM") as pfin:
        normalize_oproj(accs0, 0, attnsb, pfin, outsb, tag="fin", tbufs=2)

    # ---- Phase B: attention + o-proj for query blocks 1..3 ----
    with (
        tc.tile_pool(name="scpool", bufs=2, space="PSUM") as scpool,
        tc.tile_pool(name="accpool", bufs=4, space="PSUM") as accpool,
        tc.tile_pool(name="ptpool", bufs=8) as ptpool,
        tc.tile_pool(name="attnsb", bufs=2) as attnsb,
        tc.tile_pool(name="outsb", bufs=3) as outsb,
    ):
        for qb in range(1, NQB):
            q0 = qb * QB
            accs = [accpool.tile([128, QB], FP32, tag="acc", name="acc")
                    for _ in range(HPC)]
            for kb2 in range(NKB // 2):
                pts = []
                for h in range(HPC):
                    ps = scpool.tile([128, 2 * QB], FP32, tag="sc", name="sc")
                    for j in range(2):
                        kb = kb2 * 2 + j
                        scores_mms(ps[:, j * QB:(j + 1) * QB], h, kb, q0, QB)
                    pt = ptpool.tile([128, 2 * QB], BF16, tag="pt", name="pt")
                    nc.scalar.activation(pt[:], ps[:], Exp, scale=SCALE)
                    pts.append(pt)
                for h in range(HPC):
                    for j in range(2):
                        kb = kb2 * 2 + j
                        nc.tensor.matmul(
                            accs[h][0:65, :],
                            vhx[h][:, kb * 65:kb * 65 + 65],
                            pts[h][:, j * QB:(j + 1) * QB],
                            start=(kb == 0), stop=(kb == NKB - 1))
            normalize_oproj(accs, q0, attnsb, accpool, outsb)


def prepare(q, k, v, Wq, bq, Wk, bk, Wv, bv, Wo, bo):
    """Host-side sharding: returns (in_maps for cores 0-7, bias flags)."""
    bf = ml_dtypes.bfloat16
    qT = np.ascontiguousarray(q[0].T).astype(bf)
    kTf = np.ascontiguousarray(k[0].T).astype(bf)
    vTf = np.ascontiguousarray(v[0].T).astype(bf)
    wqT = np.ascontiguousarray(np.asarray(Wq).T).astype(bf)
    wkT = np.ascontiguousarray(np.asarray(Wk).T).astype(bf)
    wvT = np.ascontiguousarray(np.asarray(Wv).T).astype(bf)
    woT = np.ascontiguousarray(np.asarray(Wo).T).astype(bf)
    bq = np.asarray(bq, np.float32)
    bk = np.asarray(bk, np.float32)
    bv = np.asarray(bv, np.float32)
    in_maps = []
    for core in range(8):
        g, s = divmod(core, 2)
        d0, d1 = g * DH, (g + 1) * DH
        in_maps.append({
            "qTs": np.ascontiguousarray(qT[:, s * SQ:(s + 1) * SQ]),
            "kT": kTf,
            "vT": vTf,
            "wq": np.ascontiguousarray(wqT[:, d0:d1]),
            "wk": np.ascontiguousarray(wkT[:, d0:d1]),
            "wv": np.ascontiguousarray(wvT[:, d0:d1]),
            "wo": np.ascontiguousarray(woT[d0:d1, :]),
            "bq": np.ascontiguousarray(bq[d0:d1]).reshape(DH, 1),
            "bk": np.ascontiguousarray(bk[d0:d1]).reshape(DH, 1),
            "bv": np.ascontiguousarray(bv[d0:d1]).reshape(DH, 1),
        })
    flags = (bool(np.any(bq)), bool(np.any(bk)), bool(np.any(bv)))
    return in_maps, flags


def combine(results, bo):
    """Host-side unsharding: sum o-proj partials per half, concat, add bo."""
    halves = []
    for s in range(2):
        acc = None
        for g in range(4):
            o = np.asarray(results[g * 2 + s]["outT"], np.float32)
            acc = o if acc is None else acc + o
        halves.append(acc.T)
    out = np.concatenate(halves, axis=0) + np.asarray(bo, np.float32)
    return np.ascontiguousarray(out).reshape(1, SEQ, D).astype(np.float32)


def kernel(q, k, v, Wq, bq, Wk, bk, Wv, bv, Wo, bo):
    from concourse.bass_utils import run_bass_kernel_spmd

    in_maps, flags = prepare(q, k, v, Wq, bq, Wk, bk, Wv, bv, Wo, bo)
    nc = build_program(*flags)
    last_err = None
    for _attempt in range(3):
        try:
            res = run_bass_kernel_spmd(nc, in_maps, list(range(8)))
            return combine(res.results, bo)
        except Exception as e:  # transient NRT/device wedges recover on retry
            last_err = e
            try:
                import jax
                jax.clear_caches()
                jax.extend.backend.clear_backends()
            except Exception:
                pass
    raise last_err
